# revision 10
# baseline (speedup 1.0000x reference)
"""CollectAtomTriples on 8 Trainium2 NeuronCores.

For each atom a (a consecutive segment of K rows in the neighbor list),
emit all P = K*(K-1)/2 unique pairs (j < k) of its neighbor-list rows:
    idx_i_triples[a*P + p] = a
    idx_j_triples[a*P + p] = base[a] + jj[p]
    idx_k_triples[a*P + p] = base[a] + kk[p]
where base = exclusive prefix sum of per-atom counts (bincount of idx_i)
and (jj, kk) = triu_indices(K, k=1) in row-major order.

Sharding: pure data parallel over atoms — each of the 8 cores generates
the triples for n_atoms/8 consecutive atoms. The per-shard offsets are
carried in per-core input tables, so one SPMD program serves all cores.

Device kernel (per core, per 128-atom tile):
  - DVE:  out_j = tmpl_jj + base_col   (tensor_scalar add, int32)
  - DVE:  out_i = 0*tmpl + atom_col    (tensor_scalar mult+add, int32)
  - ACT:  out_k = tmpl_kk + base_col   (activation Identity + bias;
          exact: values < 2^24)
  - 3x 248KB HWDGE stores to HBM (the bound: ~37MB written per core)
"""

import numpy as np

_BUILD_CACHE = {}


def _build_module(A, P, NT):
    """Build the SPMD Bass module for A atoms/shard, P pairs, NT tiles."""
    import concourse.tile as tile
    from concourse import bacc, mybir

    dt = mybir.dt.int32
    # Bacc (not raw Bass): its compile() pass splits multi-sem waits into
    # EventSemaphore instructions — TRN2 instruction structs encode only
    # ONE sync-wait, and walrus rejects instructions carrying two.
    nc = bacc.Bacc()

    # Single merged const input (one DMA -> one semaphore: the TRN2
    # Activation instruction format only encodes ONE sync-wait, so the
    # first ACT op must not depend on two separate load DMAs).
    #   [:, 0:P)          jj row (same in every partition), int32
    #   [:, P:2P)         kk row, int32
    #   [:, 2P:2P+NT)     base col table, float32 bitcast in int32 slots
    #   [:, 2P+NT:2P+2NT) atom-id col table, float32 bitcast
    # (the HW tensor_scalar/activation scalar operand path is fp32 —
    # exact for values < 2^24; max here is ~1.6M).
    CW = 2 * P + 2 * NT
    consts = nc.declare_dram_parameter("consts", [128, CW], dt, isOutput=False)
    outi = nc.declare_dram_parameter("outi", [A, P], dt, isOutput=True)
    outj = nc.declare_dram_parameter("outj", [A, P], dt, isOutput=True)
    outk = nc.declare_dram_parameter("outk", [A, P], dt, isOutput=True)

    # G full 128-atom chunks per store: ~2MB DMAs amortize the per-DMA
    # semaphore-receipt stall each SDMA engine pays at DMA boundaries.
    G = 8

    with tile.TileContext(nc) as tc:
        with (
            tc.tile_pool(name="const", bufs=1) as cpool,
            tc.tile_pool(name="work", bufs=3) as wpool,
        ):
            consts_sb = cpool.tile([128, CW], dt)
            nc.sync.dma_start(out=consts_sb[:], in_=consts[:])
            tmpl_sb = consts_sb
            cols_f32 = consts_sb[:, 2 * P : 2 * P + 2 * NT].bitcast(
                mybir.dt.float32
            )
            zcol = cpool.tile([128, 1], mybir.dt.float32)
            nc.vector.memset(zcol[:], 0.0)

            n_full = A // 128  # full 128-atom chunks
            si = 0  # store index — alternate the two HWDGE rings
            t = 0
            while t < NT:
                gmax = min(G, n_full - t)
                if gmax >= 1:
                    a0 = t * 128
                    n = gmax * 128

                    def _store(out_t, sb_t):
                        nonlocal si
                        dram_ap = out_t[a0 : a0 + n, :].rearrange(
                            "(g p) f -> p g f", p=128
                        )
                        sb_ap = sb_t[:, 0 : gmax * P].rearrange(
                            "p (g f) -> p g f", f=P
                        )
                        eng = nc.sync if si % 2 == 0 else nc.scalar
                        eng.dma_start(out=dram_ap, in_=sb_ap)
                        si += 1

                    # Each output's chunks complete together, then its
                    # store is issued immediately (overlaps later compute).
                    tj = wpool.tile([128, G * P], dt, tag="tj")
                    tk = wpool.tile([128, G * P], dt, tag="tk")
                    ti = wpool.tile([128, G * P], dt, tag="ti")
                    for g in range(gmax):
                        tt = t + g
                        nc.vector.tensor_scalar_add(
                            tj[:, g * P : (g + 1) * P],
                            tmpl_sb[:, 0:P],
                            cols_f32[:, tt : tt + 1],
                        )
                        nc.scalar.activation(
                            tk[:, g * P : (g + 1) * P],
                            tmpl_sb[:, P : 2 * P],
                            mybir.ActivationFunctionType.Identity,
                            bias=cols_f32[:, tt : tt + 1],
                            scale=1.0,
                        )
                    _store(outj, tj)
                    _store(outk, tk)
                    for g in range(gmax):
                        tt = t + g
                        nc.vector.tensor_scalar(
                            ti[:, g * P : (g + 1) * P],
                            tmpl_sb[:, 0:P],
                            zcol[:, :],
                            cols_f32[:, NT + tt : NT + tt + 1],
                            mybir.AluOpType.mult,
                            mybir.AluOpType.add,
                        )
                    _store(outi, ti)
                    t += gmax
                else:
                    # partial tail chunk (< 128 atoms)
                    pp = A - t * 128
                    base_col = cols_f32[:pp, t : t + 1]
                    atom_col = cols_f32[:pp, NT + t : NT + t + 1]
                    tj = wpool.tile([128, G * P], dt, tag="tj")
                    tk = wpool.tile([128, G * P], dt, tag="tk")
                    ti = wpool.tile([128, G * P], dt, tag="ti")
                    nc.vector.tensor_scalar_add(
                        tj[:pp, 0:P], tmpl_sb[:pp, 0:P], base_col
                    )
                    nc.scalar.activation(
                        tk[:pp, 0:P],
                        tmpl_sb[:pp, P : 2 * P],
                        mybir.ActivationFunctionType.Identity,
                        bias=base_col,
                        scale=1.0,
                    )
                    nc.vector.tensor_scalar(
                        ti[:pp, 0:P],
                        tmpl_sb[:pp, 0:P],
                        zcol[:pp, :],
                        atom_col,
                        mybir.AluOpType.mult,
                        mybir.AluOpType.add,
                    )
                    r0 = t * 128
                    for out_t, sb_t in ((outj, tj), (outk, tk), (outi, ti)):
                        eng = nc.sync if si % 2 == 0 else nc.scalar
                        eng.dma_start(
                            out=out_t[r0 : r0 + pp, :], in_=sb_t[:pp, 0:P]
                        )
                        si += 1
                    t += 1

    nc.finalize()
    return nc


def _get_module(A, P, NT):
    key = (A, P, NT)
    if key not in _BUILD_CACHE:
        _BUILD_CACHE[key] = _build_module(A, P, NT)
    return _BUILD_CACHE[key]


def _pack_cols(vals, NT):
    """[NT*128] values -> [128, NT] table (col t = vals[t*128 : (t+1)*128])."""
    return np.ascontiguousarray(vals.reshape(NT, 128).T)


def kernel(idx_i, n_atoms, k_neighbors, _collect_timing=None):
    n_atoms = int(n_atoms)
    K = int(k_neighbors)
    P = K * (K - 1) // 2
    M = 8  # cores

    idx_i = np.asarray(idx_i, dtype=np.int32)
    counts = np.bincount(idx_i, minlength=n_atoms)[:n_atoms]
    base = (np.cumsum(counts) - counts).astype(np.int32)

    # Shard atoms: A per core, padded so every core runs the same program.
    A = -(-n_atoms // M)  # ceil
    NT = -(-A // 128)
    Apad = NT * 128

    jj, kk = np.triu_indices(K, k=1)

    base_pad = np.zeros(M * Apad, dtype=np.int32)
    atom_pad = np.zeros(M * Apad, dtype=np.int32)
    for c in range(M):
        lo = c * A
        hi = min(n_atoms, lo + A)
        base_pad[c * Apad : c * Apad + (hi - lo)] = base[lo:hi]
        atom_pad[c * Apad : c * Apad + (hi - lo)] = np.arange(
            lo, hi, dtype=np.int32
        )

    in_maps = []
    for c in range(M):
        consts = np.empty((128, 2 * P + 2 * NT), dtype=np.int32)
        consts[:, 0:P] = jj.astype(np.int32)[None, :]
        consts[:, P : 2 * P] = kk.astype(np.int32)[None, :]
        cols = np.empty((128, 2 * NT), dtype=np.float32)
        cols[:, 0:NT] = _pack_cols(base_pad[c * Apad : (c + 1) * Apad], NT)
        cols[:, NT:] = _pack_cols(atom_pad[c * Apad : (c + 1) * Apad], NT)
        consts[:, 2 * P :] = cols.view(np.int32)
        in_maps.append({"consts": consts})

    from concourse.bass_utils import run_bass_kernel_spmd

    nc = _get_module(A, P, NT)
    res = run_bass_kernel_spmd(
        nc,
        in_maps,
        list(range(M)),
        trace=_collect_timing is not None,
    )
    if _collect_timing is not None:
        _collect_timing["results"] = res

    out_i = np.empty((n_atoms, P), dtype=np.int32)
    out_j = np.empty((n_atoms, P), dtype=np.int32)
    out_k = np.empty((n_atoms, P), dtype=np.int32)
    for c in range(M):
        lo = c * A
        hi = min(n_atoms, lo + A)
        out_i[lo:hi] = res.results[c]["outi"][: hi - lo]
        out_j[lo:hi] = res.results[c]["outj"][: hi - lo]
        out_k[lo:hi] = res.results[c]["outk"][: hi - lo]

    return out_i.reshape(-1), out_j.reshape(-1), out_k.reshape(-1)


# revision 11
# speedup vs baseline: 1.0116x; 1.0116x over previous
"""CollectAtomTriples on 8 Trainium2 NeuronCores.

For each atom a (a consecutive segment of K rows in the neighbor list),
emit all P = K*(K-1)/2 unique pairs (j < k) of its neighbor-list rows:
    idx_i_triples[a*P + p] = a
    idx_j_triples[a*P + p] = base[a] + jj[p]
    idx_k_triples[a*P + p] = base[a] + kk[p]
where base = exclusive prefix sum of per-atom counts (bincount of idx_i)
and (jj, kk) = triu_indices(K, k=1) in row-major order.

Sharding: pure data parallel over atoms — each of the 8 cores generates
the triples for n_atoms/8 consecutive atoms. The per-shard offsets are
carried in per-core input tables, so one SPMD program serves all cores.

Device kernel (per core, per 128-atom tile):
  - DVE:  out_j = tmpl_jj + base_col   (tensor_scalar add, int32)
  - DVE:  out_i = 0*tmpl + atom_col    (tensor_scalar mult+add, int32)
  - ACT:  out_k = tmpl_kk + base_col   (activation Identity + bias;
          exact: values < 2^24)
  - 3x 248KB HWDGE stores to HBM (the bound: ~37MB written per core)
"""

import numpy as np

_BUILD_CACHE = {}


def _build_module(A, P, NT):
    """Build the SPMD Bass module for A atoms/shard, P pairs, NT tiles."""
    import concourse.tile as tile
    from concourse import bacc, mybir

    dt = mybir.dt.int32
    # Bacc (not raw Bass): its compile() pass splits multi-sem waits into
    # EventSemaphore instructions — TRN2 instruction structs encode only
    # ONE sync-wait, and walrus rejects instructions carrying two.
    nc = bacc.Bacc()

    # Single merged const input (one DMA -> one semaphore: the TRN2
    # Activation instruction format only encodes ONE sync-wait, so the
    # first ACT op must not depend on two separate load DMAs).
    #   [:, 0:P)          jj row (same in every partition), int32
    #   [:, P:2P)         kk row, int32
    #   [:, 2P:2P+NT)     base col table, float32 bitcast in int32 slots
    #   [:, 2P+NT:2P+2NT) atom-id col table, float32 bitcast
    # (the HW tensor_scalar/activation scalar operand path is fp32 —
    # exact for values < 2^24; max here is ~1.6M).
    CW = 2 * P + 2 * NT
    consts = nc.declare_dram_parameter("consts", [128, CW], dt, isOutput=False)
    outi = nc.declare_dram_parameter("outi", [A, P], dt, isOutput=True)
    outj = nc.declare_dram_parameter("outj", [A, P], dt, isOutput=True)
    outk = nc.declare_dram_parameter("outk", [A, P], dt, isOutput=True)

    # G full 128-atom chunks per store: ~1MB DMAs amortize the per-DMA
    # semaphore-receipt stall each SDMA engine pays at DMA boundaries.
    G = 4

    with tile.TileContext(nc) as tc:
        with (
            tc.tile_pool(name="const", bufs=1) as cpool,
            tc.tile_pool(name="work", bufs=3) as wpool,
        ):
            consts_sb = cpool.tile([128, CW], dt)
            nc.sync.dma_start(out=consts_sb[:], in_=consts[:])
            tmpl_sb = consts_sb
            cols_f32 = consts_sb[:, 2 * P : 2 * P + 2 * NT].bitcast(
                mybir.dt.float32
            )
            zcol = cpool.tile([128, 1], mybir.dt.float32)
            nc.vector.memset(zcol[:], 0.0)

            n_full = A // 128  # full 128-atom chunks
            si = 0  # store index — alternate the two HWDGE rings
            t = 0
            while t < NT:
                gmax = min(G, n_full - t)
                if gmax >= 1:
                    a0 = t * 128
                    n = gmax * 128

                    def _store(out_t, sb_t):
                        nonlocal si
                        dram_ap = out_t[a0 : a0 + n, :].rearrange(
                            "(g p) f -> p g f", p=128
                        )
                        sb_ap = sb_t[:, 0 : gmax * P].rearrange(
                            "p (g f) -> p g f", f=P
                        )
                        engs = (nc.sync, nc.scalar, nc.gpsimd)
                        engs[si % 3].dma_start(out=dram_ap, in_=sb_ap)
                        si += 1

                    tj = wpool.tile([128, G * P], dt, tag="tj")
                    tk = wpool.tile([128, G * P], dt, tag="tk")
                    ti = wpool.tile([128, G * P], dt, tag="ti")
                    for g in range(gmax):
                        tt = t + g
                        base_col = cols_f32[:, tt : tt + 1]
                        atom_col = cols_f32[:, NT + tt : NT + tt + 1]
                        c0 = g * P
                        nc.vector.tensor_scalar_add(
                            tj[:, c0 : c0 + P], tmpl_sb[:, 0:P], base_col
                        )
                        nc.scalar.activation(
                            tk[:, c0 : c0 + P],
                            tmpl_sb[:, P : 2 * P],
                            mybir.ActivationFunctionType.Identity,
                            bias=base_col,
                            scale=1.0,
                        )
                        nc.vector.tensor_scalar(
                            ti[:, c0 : c0 + P],
                            tmpl_sb[:, 0:P],
                            zcol[:, :],
                            atom_col,
                            mybir.AluOpType.mult,
                            mybir.AluOpType.add,
                        )
                    _store(outj, tj)
                    _store(outk, tk)
                    _store(outi, ti)
                    t += gmax
                else:
                    # partial tail chunk (< 128 atoms)
                    pp = A - t * 128
                    base_col = cols_f32[:pp, t : t + 1]
                    atom_col = cols_f32[:pp, NT + t : NT + t + 1]
                    tj = wpool.tile([128, G * P], dt, tag="tj")
                    tk = wpool.tile([128, G * P], dt, tag="tk")
                    ti = wpool.tile([128, G * P], dt, tag="ti")
                    nc.vector.tensor_scalar_add(
                        tj[:pp, 0:P], tmpl_sb[:pp, 0:P], base_col
                    )
                    nc.scalar.activation(
                        tk[:pp, 0:P],
                        tmpl_sb[:pp, P : 2 * P],
                        mybir.ActivationFunctionType.Identity,
                        bias=base_col,
                        scale=1.0,
                    )
                    nc.vector.tensor_scalar(
                        ti[:pp, 0:P],
                        tmpl_sb[:pp, 0:P],
                        zcol[:pp, :],
                        atom_col,
                        mybir.AluOpType.mult,
                        mybir.AluOpType.add,
                    )
                    r0 = t * 128
                    for out_t, sb_t in ((outj, tj), (outk, tk), (outi, ti)):
                        engs = (nc.sync, nc.scalar, nc.gpsimd)
                        engs[si % 3].dma_start(
                            out=out_t[r0 : r0 + pp, :], in_=sb_t[:pp, 0:P]
                        )
                        si += 1
                    t += 1

    nc.finalize()
    return nc


def _get_module(A, P, NT):
    key = (A, P, NT)
    if key not in _BUILD_CACHE:
        _BUILD_CACHE[key] = _build_module(A, P, NT)
    return _BUILD_CACHE[key]


def _pack_cols(vals, NT):
    """[NT*128] values -> [128, NT] table (col t = vals[t*128 : (t+1)*128])."""
    return np.ascontiguousarray(vals.reshape(NT, 128).T)


def kernel(idx_i, n_atoms, k_neighbors, _collect_timing=None):
    n_atoms = int(n_atoms)
    K = int(k_neighbors)
    P = K * (K - 1) // 2
    M = 8  # cores

    idx_i = np.asarray(idx_i, dtype=np.int32)
    counts = np.bincount(idx_i, minlength=n_atoms)[:n_atoms]
    base = (np.cumsum(counts) - counts).astype(np.int32)

    # Shard atoms: A per core, padded so every core runs the same program.
    A = -(-n_atoms // M)  # ceil
    NT = -(-A // 128)
    Apad = NT * 128

    jj, kk = np.triu_indices(K, k=1)

    base_pad = np.zeros(M * Apad, dtype=np.int32)
    atom_pad = np.zeros(M * Apad, dtype=np.int32)
    for c in range(M):
        lo = c * A
        hi = min(n_atoms, lo + A)
        base_pad[c * Apad : c * Apad + (hi - lo)] = base[lo:hi]
        atom_pad[c * Apad : c * Apad + (hi - lo)] = np.arange(
            lo, hi, dtype=np.int32
        )

    in_maps = []
    for c in range(M):
        consts = np.empty((128, 2 * P + 2 * NT), dtype=np.int32)
        consts[:, 0:P] = jj.astype(np.int32)[None, :]
        consts[:, P : 2 * P] = kk.astype(np.int32)[None, :]
        cols = np.empty((128, 2 * NT), dtype=np.float32)
        cols[:, 0:NT] = _pack_cols(base_pad[c * Apad : (c + 1) * Apad], NT)
        cols[:, NT:] = _pack_cols(atom_pad[c * Apad : (c + 1) * Apad], NT)
        consts[:, 2 * P :] = cols.view(np.int32)
        in_maps.append({"consts": consts})

    from concourse.bass_utils import run_bass_kernel_spmd

    nc = _get_module(A, P, NT)
    res = run_bass_kernel_spmd(
        nc,
        in_maps,
        list(range(M)),
        trace=_collect_timing is not None,
    )
    if _collect_timing is not None:
        _collect_timing["results"] = res

    out_i = np.empty((n_atoms, P), dtype=np.int32)
    out_j = np.empty((n_atoms, P), dtype=np.int32)
    out_k = np.empty((n_atoms, P), dtype=np.int32)
    for c in range(M):
        lo = c * A
        hi = min(n_atoms, lo + A)
        out_i[lo:hi] = res.results[c]["outi"][: hi - lo]
        out_j[lo:hi] = res.results[c]["outj"][: hi - lo]
        out_k[lo:hi] = res.results[c]["outk"][: hi - lo]

    return out_i.reshape(-1), out_j.reshape(-1), out_k.reshape(-1)
